# revision 7
# baseline (speedup 1.0000x reference)
"""Trainium2 Bass kernel for nn_MixFormerBlock (8 NeuronCores, data-parallel over batch).

Strategy (per core, 2 of 16 batches => 2048 tokens):
  - seq FFN (dominant: 3 x [2048x2048]@[2048x8192]-class matmuls) in bf16 on PE:
      rmsnorm -> sn (bf16, spilled to HBM) -> per token-quarter: DMA-transpose to snT,
      G/V matmuls (weights stationary, tokens moving), silu*val -> H^T resident in SBUF,
      out matmuls (H^T stationary, Wo moving) -> +residual -> seq_hidden (fp32 out,
      bf16 copy spilled to HBM for attention).
  - attention: per head, DMA-transpose seq_hidden head-slice, keys^T/vals matmuls,
    scores^T via PE (token-partition layout), exp on ACT, denominators via ones-matmul,
    z^T accumulated into one PSUM tile, PE-transpose back.
  - QueryMixer / OutputFusion small per-head swiglus batched over (batch, head) columns.
No collectives: pure SPMD data parallelism; host gathers per-core outputs.
"""

import os
import sys

for _p in ("/opt/trn_rl_repo", "/root/.axon_site/_ro/trn_rl_repo"):
    if os.path.isdir(_p) and _p not in sys.path:
        sys.path.insert(0, _p)

import math
import numpy as np
import ml_dtypes

import concourse.bass as bass
import concourse.mybir as mybir
import concourse.tile as tile
from concourse import bacc
from concourse.bass_utils import run_bass_kernel_spmd
from concourse.masks import make_identity

F32 = mybir.dt.float32
BF16 = mybir.dt.bfloat16
AX = mybir.AxisListType.X
AF = mybir.ActivationFunctionType
OP = mybir.AluOpType
EPS = 1e-8


class Cfg:
    def __init__(self, BL, T, D, H, HF, HQ, NU):
        self.BL = BL          # local batches per core
        self.T = T            # seq len per batch
        self.D = D            # model dim (= H*128)
        self.H = H            # heads
        self.HF = HF          # seq FFN hidden
        self.HQ = HQ          # per-head FFN hidden
        self.NU = NU          # num user heads (mask only, host side)
        self.HD = 128
        self.TT = BL * T      # local tokens
        self.KC = D // 128    # D chunks
        self.NCF = HF // 128  # HF chunks
        self.KQ = HQ // 128   # HQ chunks
        self.BH = BL * H
        self.TCB = T // 128   # token chunks per batch
        self.GTC = self.TT // 128
        self.TQ = min(512, self.TT)   # tokens per FFN chunk ("quarter")
        self.NQ = self.TT // self.TQ
        self.DHW = min(512, D)        # out-matmul rhs width
        self.NDH = D // self.DHW
        assert self.TQ % 512 == 0 or self.TQ == self.TT
        assert D == H * 128


FULL = Cfg(BL=2, T=1024, D=2048, H=16, HF=8192, HQ=512, NU=4)
N_CORES = 8
B_FULL, T_FULL, D_FULL = 16, 1024, 2048


def _rms_small(nc, pool, psum, x_sb, w_rep, eps_col, out_dt, name):
    """rmsnorm over free dim (128) of x_sb [P,128] f32; returns [P,128] out_dt."""
    P = x_sb.shape[0]
    sq = pool.tile([P, 128], F32, name=f"{name}_sq")
    nc.vector.tensor_mul(sq, x_sb, x_sb)
    s = pool.tile([P, 1], F32, name=f"{name}_s")
    nc.vector.reduce_sum(s, sq, axis=AX)
    nc.scalar.activation(s, s, AF.Sqrt, bias=eps_col[:P], scale=1.0 / 128)
    nc.vector.reciprocal(s, s)
    tmp = pool.tile([P, 128], F32, name=f"{name}_tmp")
    nc.vector.tensor_scalar_mul(tmp, x_sb, s)
    out = pool.tile([P, 128], out_dt, name=f"{name}_out")
    nc.vector.tensor_tensor(out, tmp, w_rep[:P], OP.mult)
    return out


def _swiglu_T(nc, c, pool, psum, idn_f32, xT_bf, gw_sb, vw_sb, ow_sb, gbT, vbT, name):
    """Per-head swiglu, batched over (b,h) columns.

    xT_bf: [128 d, BH] bf16 (columns ordered b*H+h).
    gw_sb/vw_sb: [128 d, H, HQ] bf16;  ow_sb: [128 k, H, KQ, 128] bf16.
    gbT/vbT: [128 k, KQ, H] f32.
    Returns osw rows [BH, 128] f32 in PSUM (caller consumes immediately).
    """
    KQ, H, BL, BH = c.KQ, c.H, c.BL, c.BH
    gT = pool.tile([128, KQ, BL, H], F32, name=f"{name}_gT")
    vT = pool.tile([128, KQ, BL, H], F32, name=f"{name}_vT")
    xT_by_h = xT_bf.rearrange("p (b h) -> p h b", h=H)
    for h in range(H):
        gv_ps = psum.tile([128, 2, KQ, BL], F32, name=f"{name}_gvps")
        for kc in range(KQ):
            nc.tensor.matmul(gv_ps[:, 0, kc, :], lhsT=gw_sb[:, h, kc * 128:(kc + 1) * 128],
                             rhs=xT_by_h[:, h, :], start=True, stop=True)
            nc.tensor.matmul(gv_ps[:, 1, kc, :], lhsT=vw_sb[:, h, kc * 128:(kc + 1) * 128],
                             rhs=xT_by_h[:, h, :], start=True, stop=True)
        nc.vector.tensor_copy(gT[:, :, :, h], gv_ps[:, 0, :, :])
        nc.vector.tensor_copy(vT[:, :, :, h], gv_ps[:, 1, :, :])
    shp = [128, KQ, BL, H]
    nc.vector.tensor_tensor(gT, gT, gbT[:, :, None, :].to_broadcast(shp), OP.add)
    gs = pool.tile(shp, BF16, name=f"{name}_gs")
    nc.scalar.activation(gs, gT, AF.Silu)
    vs = pool.tile(shp, BF16, name=f"{name}_vs")
    nc.vector.tensor_tensor(vs, vT, vbT[:, :, None, :].to_broadcast(shp), OP.add)
    hT = pool.tile(shp, BF16, name=f"{name}_hT")
    nc.vector.tensor_mul(hT, gs, vs)
    # out matmuls, transposed: oT[e, bh] accumulated per column
    oT_ps = psum.tile([128, BH], F32, name=f"{name}_oTps")
    for h in range(H):
        for kc in range(KQ):
            for b in range(BL):
                nc.tensor.matmul(oT_ps[:, b * H + h: b * H + h + 1],
                                 lhsT=ow_sb[:, h, kc, :],
                                 rhs=hT[:, kc, b, h:h + 1],
                                 start=(kc == 0), stop=(kc == KQ - 1),
                                 skip_group_check=True)
    oT_sb = pool.tile([128, BH], F32, name=f"{name}_oTsb")
    nc.vector.tensor_copy(oT_sb, oT_ps)
    o_ps = psum.tile([BH, 128], F32, name=f"{name}_ops")
    nc.tensor.transpose(o_ps, oT_sb, idn_f32)
    return o_ps


def build_nc(c: Cfg):
    nc = bacc.Bacc(None, target_bir_lowering=False)

    # ---- DRAM I/O ----
    xseq = nc.dram_tensor("xseq", [c.TT, c.D], F32, kind="ExternalInput")[:]
    xh = nc.dram_tensor("xh", [c.BH, 128], F32, kind="ExternalInput")[:]
    maskT = nc.dram_tensor("maskT", [128, c.TCB, c.BL], F32, kind="ExternalInput")[:]
    wg = nc.dram_tensor("wg", [c.D, c.HF], BF16, kind="ExternalInput")[:]
    wv = nc.dram_tensor("wv", [c.D, c.HF], BF16, kind="ExternalInput")[:]
    wo = nc.dram_tensor("wo", [c.HF, c.D], BF16, kind="ExternalInput")[:]
    sfgb = nc.dram_tensor("sfgb", [c.HF], F32, kind="ExternalInput")[:]
    sfvb = nc.dram_tensor("sfvb", [c.HF], F32, kind="ExternalInput")[:]
    sfob = nc.dram_tensor("sfob", [c.D], F32, kind="ExternalInput")[:]
    sfnw = nc.dram_tensor("sfnw", [c.D], F32, kind="ExternalInput")[:]
    kw = nc.dram_tensor("kw", [c.H, 128, 128], BF16, kind="ExternalInput")[:]
    vw = nc.dram_tensor("vw", [c.H, 128, 128], BF16, kind="ExternalInput")[:]
    kb = nc.dram_tensor("kb", [c.H, 128], F32, kind="ExternalInput")[:]
    vb = nc.dram_tensor("vb", [c.H, 128], F32, kind="ExternalInput")[:]
    # qm / of small weights
    qnin = nc.dram_tensor("qnin", [128], F32, kind="ExternalInput")[:]
    qnh = nc.dram_tensor("qnh", [128], F32, kind="ExternalInput")[:]
    onw = nc.dram_tensor("onw", [128], F32, kind="ExternalInput")[:]
    um = nc.dram_tensor("um", [c.BH, 128], F32, kind="ExternalInput")[:]
    qgw = nc.dram_tensor("qgw", [c.H, 128, c.HQ], BF16, kind="ExternalInput")[:]
    qvw = nc.dram_tensor("qvw", [c.H, 128, c.HQ], BF16, kind="ExternalInput")[:]
    qow = nc.dram_tensor("qow", [c.H, c.HQ, 128], BF16, kind="ExternalInput")[:]
    qgbT = nc.dram_tensor("qgbT", [128, c.KQ, c.H], F32, kind="ExternalInput")[:]
    qvbT = nc.dram_tensor("qvbT", [128, c.KQ, c.H], F32, kind="ExternalInput")[:]
    qob = nc.dram_tensor("qob", [c.BH, 128], F32, kind="ExternalInput")[:]
    ogw = nc.dram_tensor("ogw", [c.H, 128, c.HQ], BF16, kind="ExternalInput")[:]
    ovw = nc.dram_tensor("ovw", [c.H, 128, c.HQ], BF16, kind="ExternalInput")[:]
    oow = nc.dram_tensor("oow", [c.H, c.HQ, 128], BF16, kind="ExternalInput")[:]
    ogbT = nc.dram_tensor("ogbT", [128, c.KQ, c.H], F32, kind="ExternalInput")[:]
    ovbT = nc.dram_tensor("ovbT", [128, c.KQ, c.H], F32, kind="ExternalInput")[:]
    oob = nc.dram_tensor("oob", [c.BH, 128], F32, kind="ExternalInput")[:]

    seqh = nc.dram_tensor("seqh", [c.TT, c.D], F32, kind="ExternalOutput")[:]
    oout = nc.dram_tensor("oout", [c.BH, 128], F32, kind="ExternalOutput")[:]

    with tile.TileContext(nc) as tc:
        with tc.tile_pool(name="dram", bufs=1, space="DRAM") as dram, \
             tc.tile_pool(name="singles", bufs=1) as sg:

            sn_dram = dram.tile([c.TT, c.D], BF16)
            shb_dram = dram.tile([c.TT, c.D], BF16)
            qm_dram = dram.tile([c.BH, 128], F32)
            rinv_dram = dram.tile([c.BL, c.H], F32)

            # ---- persistent small tiles ----
            idn_bf = sg.tile([128, 128], BF16)
            make_identity(nc, idn_bf)
            idn_f32 = sg.tile([128, 128], F32)
            make_identity(nc, idn_f32)
            eps_col = sg.tile([128, 1], F32)
            nc.vector.memset(eps_col, EPS)
            ones_bf = sg.tile([128, 1], BF16)
            nc.vector.memset(ones_bf, 1.0)

            sfnw_sb = sg.tile([128, c.D], F32)
            nc.sync.dma_start(out=sfnw_sb, in_=sfnw[None, :].to_broadcast([128, c.D]))
            sfob_sb = sg.tile([128, c.D], F32)
            nc.sync.dma_start(out=sfob_sb, in_=sfob[None, :].to_broadcast([128, c.D]))
            gb_sb = sg.tile([128, c.NCF], F32)
            nc.sync.dma_start(out=gb_sb, in_=sfgb.rearrange("(n p) -> p n", p=128))
            vbf_sb = sg.tile([128, c.NCF], F32)
            nc.sync.dma_start(out=vbf_sb, in_=sfvb.rearrange("(n p) -> p n", p=128))

            kw_sb = sg.tile([128, c.H, 128], BF16)
            nc.sync.dma_start(out=kw_sb, in_=kw.rearrange("h d e -> d h e"))
            vw_sb = sg.tile([128, c.H, 128], BF16)
            nc.sync.dma_start(out=vw_sb, in_=vw.rearrange("h d e -> d h e"))
            kb_sb = sg.tile([128, c.H], F32)
            nc.sync.dma_start(out=kb_sb, in_=kb.rearrange("h e -> e h"))
            vbh_sb = sg.tile([128, c.H, 128], F32)
            nc.sync.dma_start(out=vbh_sb, in_=vb[None, :, :].to_broadcast([128, c.H, 128]))
            maskT_sb = sg.tile([128, c.TCB, c.BL], F32)
            nc.sync.dma_start(out=maskT_sb, in_=maskT)

            qnin_sb = sg.tile([128, 128], F32)
            nc.sync.dma_start(out=qnin_sb, in_=qnin[None, :].to_broadcast([128, 128]))
            qnh_sb = sg.tile([128, 128], F32)
            nc.sync.dma_start(out=qnh_sb, in_=qnh[None, :].to_broadcast([128, 128]))
            onw_sb = sg.tile([128, 128], F32)
            nc.sync.dma_start(out=onw_sb, in_=onw[None, :].to_broadcast([128, 128]))
            um_sb = sg.tile([c.BH, 128], F32)
            nc.sync.dma_start(out=um_sb, in_=um)
            qob_sb = sg.tile([c.BH, 128], F32)
            nc.sync.dma_start(out=qob_sb, in_=qob)
            oob_sb = sg.tile([c.BH, 128], F32)
            nc.sync.dma_start(out=oob_sb, in_=oob)

            # persistent cross-phase results
            q_all = sg.tile([c.BH, 128], F32)       # q (unscaled)
            qTs = sg.tile([128, c.BH], BF16)        # q^T * 1/sqrt(128)
            z_all = sg.tile([c.BH, 128], F32)       # z = attn@vals + q
            expT = sg.tile([128, c.BL, c.TCB, c.H], BF16)
            rinv = sg.tile([1, c.BL, c.H], F32)

            # ================= QueryMixer =================
            with tc.tile_pool(name="qmw", bufs=1) as qmw, \
                 tc.tile_pool(name="qmp", bufs=2) as qmp, \
                 tc.tile_pool(name="qmps", bufs=1, space="PSUM") as qmps:
                qgw_sb = qmw.tile([128, c.H, c.HQ], BF16)
                nc.sync.dma_start(out=qgw_sb, in_=qgw.rearrange("h d k -> d h k"))
                qvw_sb = qmw.tile([128, c.H, c.HQ], BF16)
                nc.sync.dma_start(out=qvw_sb, in_=qvw.rearrange("h d k -> d h k"))
                qow_sb = qmw.tile([128, c.H, c.KQ, 128], BF16)
                nc.sync.dma_start(out=qow_sb, in_=qow.rearrange("h (kc p) e -> p h kc e", p=128))
                qgbT_sb = qmw.tile([128, c.KQ, c.H], F32)
                nc.sync.dma_start(out=qgbT_sb, in_=qgbT)
                qvbT_sb = qmw.tile([128, c.KQ, c.H], F32)
                nc.sync.dma_start(out=qvbT_sb, in_=qvbT)

                xh_sb = qmp.tile([c.BH, 128], F32, name="xh_sb")
                nc.sync.dma_start(out=xh_sb, in_=xh)
                nin = _rms_small(nc, qmp, qmps, xh_sb, qnin_sb, eps_col, F32, "qmn")
                nc.sync.dma_start(out=qm_dram[:], in_=nin)
                mixed = qmp.tile([c.BH, 128], F32, name="mixed")
                for b in range(c.BL):
                    nc.sync.dma_start(
                        out=mixed[b * c.H:(b + 1) * c.H].rearrange("p (j ch) -> p j ch", j=c.H),
                        in_=qm_dram[b * c.H:(b + 1) * c.H].rearrange("j (i ch) -> i j ch", i=c.H),
                    )
                p_sb = qmp.tile([c.BH, 128], F32, name="p_sb")
                nc.vector.tensor_mul(p_sb, mixed, um_sb)
                nc.vector.tensor_add(p_sb, p_sb, xh_sb)
                pn = _rms_small(nc, qmp, qmps, p_sb, qnh_sb, eps_col, BF16, "qmh")
                pnT_ps = qmps.tile([128, c.BH], BF16, name="pnT_ps")
                nc.tensor.transpose(pnT_ps, pn, idn_bf[:c.BH, :c.BH])
                pnT = qmp.tile([128, c.BH], BF16, name="pnT")
                nc.vector.tensor_copy(pnT, pnT_ps)
                osw_ps = _swiglu_T(nc, c, qmp, qmps, idn_f32, pnT,
                                   qgw_sb, qvw_sb, qow_sb, qgbT_sb, qvbT_sb, "qm")
                nc.vector.tensor_add(q_all, osw_ps, p_sb)
                nc.vector.tensor_add(q_all, q_all, qob_sb)
                qs_bf = qmp.tile([c.BH, 128], BF16, name="qs_bf")
                nc.vector.tensor_scalar_mul(qs_bf, q_all, 1.0 / math.sqrt(128.0))
                qTs_ps = qmps.tile([128, c.BH], BF16, name="qTs_ps")
                nc.tensor.transpose(qTs_ps, qs_bf, idn_bf[:c.BH, :c.BH])
                nc.vector.tensor_copy(qTs, qTs_ps)

            # ================= seq FFN =================
            # Phase R: rmsnorm -> sn_dram (bf16)
            with tc.tile_pool(name="rp", bufs=3) as rp:
                for tt in range(c.GTC):
                    xt = rp.tile([128, c.D], F32, name="r_xt")
                    nc.sync.dma_start(out=xt, in_=xseq[tt * 128:(tt + 1) * 128, :])
                    sq = rp.tile([128, c.D], F32, name="r_sq")
                    nc.vector.tensor_mul(sq, xt, xt)
                    s = rp.tile([128, 1], F32, name="r_s")
                    nc.vector.reduce_sum(s, sq, axis=AX)
                    nc.scalar.activation(s, s, AF.Sqrt, bias=eps_col, scale=1.0 / c.D)
                    nc.vector.reciprocal(s, s)
                    tmp = rp.tile([128, c.D], F32, name="r_tmp")
                    nc.vector.tensor_scalar_mul(tmp, xt, s)
                    snt = rp.tile([128, c.D], BF16, name="r_snt")
                    nc.vector.tensor_tensor(snt, tmp, sfnw_sb, OP.mult)
                    nc.sync.dma_start(out=sn_dram[tt * 128:(tt + 1) * 128, :], in_=snt)

            # Quarters
            with tc.tile_pool(name="ffw", bufs=2) as ffw, \
                 tc.tile_pool(name="ffs", bufs=2) as ffs, \
                 tc.tile_pool(name="ffh", bufs=1) as ffh, \
                 tc.tile_pool(name="ffe", bufs=3) as ffe, \
                 tc.tile_pool(name="gvps", bufs=2, space="PSUM") as gvps, \
                 tc.tile_pool(name="outps", bufs=1, space="PSUM") as outps:

                Hq = ffh.tile([128, c.NCF, c.TQ], BF16)
                wg_r = wg.rearrange("(kc p) f -> p kc f", p=128)
                wv_r = wv.rearrange("(kc p) f -> p kc f", p=128)
                wo_r = wo.rearrange("(kc p) d -> p kc d", p=128)

                for q in range(c.NQ):
                    t0 = q * c.TQ
                    # transpose sn chunk -> snT_q
                    snT_q = ffs.tile([128, c.KC, c.TQ], BF16, name="snT_q")
                    for dc in range(c.KC):
                        nc.sync.dma_start_transpose(
                            snT_q[:, dc, :],
                            sn_dram[t0:t0 + c.TQ, dc * 128:(dc + 1) * 128])
                    # G/V + H
                    for ncf in range(c.NCF):
                        wg_sl = ffw.tile([128, c.KC, 128], BF16, name="wg_sl")
                        nc.sync.dma_start(out=wg_sl, in_=wg_r[:, :, ncf * 128:(ncf + 1) * 128])
                        wv_sl = ffw.tile([128, c.KC, 128], BF16, name="wv_sl")
                        nc.sync.dma_start(out=wv_sl, in_=wv_r[:, :, ncf * 128:(ncf + 1) * 128])
                        g_ps = gvps.tile([128, c.TQ], F32, name="g_ps")
                        for kc in range(c.KC):
                            nc.tensor.matmul(g_ps, lhsT=wg_sl[:, kc, :], rhs=snT_q[:, kc, :],
                                             start=(kc == 0), stop=(kc == c.KC - 1))
                        gs = ffe.tile([128, c.TQ], BF16, name="gs")
                        nc.scalar.activation(gs, g_ps, AF.Silu,
                                             bias=gb_sb[:, ncf:ncf + 1], scale=1.0)
                        v_ps = gvps.tile([128, c.TQ], F32, name="v_ps")
                        for kc in range(c.KC):
                            nc.tensor.matmul(v_ps, lhsT=wv_sl[:, kc, :], rhs=snT_q[:, kc, :],
                                             start=(kc == 0), stop=(kc == c.KC - 1))
                        vs = ffe.tile([128, c.TQ], BF16, name="vs")
                        nc.vector.tensor_scalar(vs, v_ps, scalar1=vbf_sb[:, ncf:ncf + 1],
                                                scalar2=None, op0=OP.add)
                        nc.vector.tensor_mul(Hq[:, ncf, :], gs, vs)
                    # out matmuls: psum [128t, DHW] per token-tile, accumulate over HF
                    NTC = c.TQ // 128
                    for dh in range(c.NDH):
                        d0 = dh * c.DHW
                        out_tiles = []
                        for tcq in range(NTC):
                            out_tiles.append(outps.tile([128, c.DHW], F32, name=f"out_ps{tcq}"))
                        for kc in range(c.NCF):
                            wo_sl = ffw.tile([128, c.DHW], BF16, name="wo_sl")
                            nc.sync.dma_start(out=wo_sl, in_=wo_r[:, kc, d0:d0 + c.DHW])
                            for tcq in range(NTC):
                                nc.tensor.matmul(out_tiles[tcq],
                                                 lhsT=Hq[:, kc, tcq * 128:(tcq + 1) * 128],
                                                 rhs=wo_sl,
                                                 start=(kc == 0), stop=(kc == c.NCF - 1))
                        for tcq in range(NTC):
                            rows = slice(t0 + tcq * 128, t0 + (tcq + 1) * 128)
                            xre = ffe.tile([128, c.DHW], F32, name="xre")
                            nc.sync.dma_start(out=xre, in_=xseq[rows, d0:d0 + c.DHW])
                            t1 = ffe.tile([128, c.DHW], F32, name="t1")
                            nc.vector.tensor_add(t1, out_tiles[tcq], xre)
                            nc.vector.tensor_tensor(
                                t1, t1, sfob_sb[:, d0:d0 + c.DHW], OP.add)
                            nc.sync.dma_start(out=seqh[rows, d0:d0 + c.DHW], in_=t1)
                            shb = ffe.tile([128, c.DHW], BF16, name="shb")
                            nc.scalar.activation(shb, t1, AF.Copy)
                            nc.sync.dma_start(out=shb_dram[rows, d0:d0 + c.DHW], in_=shb)

            # ================= attention =================
            with tc.tile_pool(name="atv", bufs=1) as atv, \
                 tc.tile_pool(name="atp", bufs=2) as atp, \
                 tc.tile_pool(name="ktps", bufs=2, space="PSUM") as ktps, \
                 tc.tile_pool(name="vaps", bufs=2, space="PSUM") as vaps, \
                 tc.tile_pool(name="stps", bufs=1, space="PSUM") as stps, \
                 tc.tile_pool(name="tailps", bufs=1, space="PSUM") as tailps:

                KTW = min(1024, c.TT)
                vals_sb = atv.tile([128, c.GTC, c.H, 128], BF16)
                for h in range(c.H):
                    shT_h = atp.tile([128, c.TT], BF16, name="shT_h")
                    nc.sync.dma_start_transpose(shT_h, shb_dram[:, h * 128:(h + 1) * 128])
                    # keys^T (in KTW-wide chunks to bound PSUM)
                    kt_sb = atp.tile([128, c.TT], BF16, name="kt_sb")
                    for kh in range(c.TT // KTW):
                        kt_ps = ktps.tile([128, KTW], F32, name="kt_ps")
                        for ts_ in range(KTW // 512):
                            o0 = ts_ * 512
                            nc.tensor.matmul(kt_ps[:, o0:o0 + 512],
                                             lhsT=kw_sb[:, h, :],
                                             rhs=shT_h[:, kh * KTW + o0:kh * KTW + o0 + 512],
                                             start=True, stop=True)
                        nc.vector.tensor_scalar(kt_sb[:, kh * KTW:(kh + 1) * KTW], kt_ps,
                                                scalar1=kb_sb[:, h:h + 1],
                                                scalar2=None, op0=OP.add)
                    # vals
                    for g in range(c.GTC):
                        va_ps = vaps.tile([128, 128], F32, name="va_ps")
                        nc.tensor.matmul(va_ps, lhsT=shT_h[:, g * 128:(g + 1) * 128],
                                         rhs=vw_sb[:, h, :], start=True, stop=True)
                        nc.vector.tensor_tensor(
                            vals_sb[:, g, h, :], va_ps, vbh_sb[:, h, :], OP.add)
                    # scores^T: [128 t, BL, TCB]
                    st_ps = stps.tile([128, c.BL, c.TCB], F32, name="st_ps")
                    for tcb in range(c.TCB):
                        for b in range(c.BL):
                            g = b * c.TCB + tcb
                            nc.tensor.matmul(st_ps[:, b, tcb:tcb + 1],
                                             lhsT=kt_sb[:, g * 128:(g + 1) * 128],
                                             rhs=qTs[:, b * c.H + h:b * c.H + h + 1],
                                             start=True, stop=True)
                    st_sb = atp.tile([128, c.BL, c.TCB], F32, name="st_sb")
                    nc.vector.tensor_tensor(st_sb, st_ps,
                                            maskT_sb.rearrange("p tc b -> p b tc"), OP.add)
                    nc.scalar.activation(expT[:, :, :, h], st_sb, AF.Exp)

                # denominators
                den_ps = tailps.tile([1, c.BL * c.TCB * c.H], F32, name="den_ps", tag="tail")
                nc.tensor.matmul(den_ps, lhsT=ones_bf,
                                 rhs=expT.rearrange("p b tc h -> p (b tc h)"),
                                 start=True, stop=True)
                den_sb = atp.tile([1, c.BL, c.TCB, c.H], F32, name="den_sb")
                nc.vector.tensor_copy(den_sb, den_ps.rearrange("p (b tc h) -> p b tc h",
                                                               b=c.BL, tc=c.TCB))
                rsum = atp.tile([1, c.BL, c.H], F32, name="rsum")
                nc.vector.reduce_sum(rsum, den_sb.rearrange("p b tc h -> p b h tc"), axis=AX)
                nc.vector.reciprocal(rinv, rsum)
                nc.sync.dma_start(out=rinv_dram[:], in_=rinv)
                rinv_rep = atp.tile([128, c.BL, c.H], F32, name="rinv_rep")
                nc.sync.dma_start(out=rinv_rep,
                                  in_=rinv_dram[None, :, :].to_broadcast([128, c.BL, c.H]))
                nc.vector.tensor_tensor(
                    expT, expT,
                    rinv_rep[:, :, None, :].to_broadcast([128, c.BL, c.TCB, c.H]), OP.mult)

                # z^T accumulated per (b,h) column
                zT_ps = tailps.tile([128, c.BH], F32, name="zT_ps", tag="tail")
                for h in range(c.H):
                    for b in range(c.BL):
                        for tcb in range(c.TCB):
                            g = b * c.TCB + tcb
                            nc.tensor.matmul(zT_ps[:, b * c.H + h:b * c.H + h + 1],
                                             lhsT=vals_sb[:, g, h, :],
                                             rhs=expT[:, b, tcb, h:h + 1],
                                             start=(tcb == 0), stop=(tcb == c.TCB - 1),
                                             skip_group_check=True)
                zT_sb = atp.tile([128, c.BH], F32, name="zT_sb")
                nc.vector.tensor_copy(zT_sb, zT_ps)
                z_ps = tailps.tile([c.BH, 128], F32, name="z_ps", tag="tail")
                nc.tensor.transpose(z_ps, zT_sb, idn_f32)
                nc.vector.tensor_add(z_all, z_ps, q_all)

            # ================= OutputFusion =================
            with tc.tile_pool(name="ofw", bufs=1) as ofw, \
                 tc.tile_pool(name="ofp", bufs=2) as ofp, \
                 tc.tile_pool(name="ofps", bufs=1, space="PSUM") as ofps:
                ogw_sb = ofw.tile([128, c.H, c.HQ], BF16)
                nc.sync.dma_start(out=ogw_sb, in_=ogw.rearrange("h d k -> d h k"))
                ovw_sb = ofw.tile([128, c.H, c.HQ], BF16)
                nc.sync.dma_start(out=ovw_sb, in_=ovw.rearrange("h d k -> d h k"))
                oow_sb = ofw.tile([128, c.H, c.KQ, 128], BF16)
                nc.sync.dma_start(out=oow_sb, in_=oow.rearrange("h (kc p) e -> p h kc e", p=128))
                ogbT_sb = ofw.tile([128, c.KQ, c.H], F32)
                nc.sync.dma_start(out=ogbT_sb, in_=ogbT)
                ovbT_sb = ofw.tile([128, c.KQ, c.H], F32)
                nc.sync.dma_start(out=ovbT_sb, in_=ovbT)

                zn = _rms_small(nc, ofp, ofps, z_all, onw_sb, eps_col, BF16, "ofn")
                znT_ps = ofps.tile([128, c.BH], BF16, name="znT_ps")
                nc.tensor.transpose(znT_ps, zn, idn_bf[:c.BH, :c.BH])
                znT = ofp.tile([128, c.BH], BF16, name="znT")
                nc.vector.tensor_copy(znT, znT_ps)
                osw_ps = _swiglu_T(nc, c, ofp, ofps, idn_f32, znT,
                                   ogw_sb, ovw_sb, oow_sb, ogbT_sb, ovbT_sb, "of")
                o_sb = ofp.tile([c.BH, 128], F32, name="o_sb")
                nc.vector.tensor_add(o_sb, osw_ps, z_all)
                nc.vector.tensor_add(o_sb, o_sb, oob_sb)
                nc.sync.dma_start(out=oout, in_=o_sb)

    nc.finalize()
    return nc


# ---------------- host side ----------------

def _bf16(a):
    return np.asarray(a, dtype=np.float32).astype(ml_dtypes.bfloat16)


def _f32(a):
    return np.ascontiguousarray(np.asarray(a, dtype=np.float32))


def _biasT(b, c):
    # [H, HQ] -> [128, KQ, H]
    b = np.asarray(b, np.float32)
    return np.ascontiguousarray(b.T.reshape(c.KQ, 128, c.H).transpose(1, 0, 2))


def _ui_mask(c):
    CH = 128 // c.H
    m = np.ones((c.H, 128), dtype=np.float32)
    m[:c.NU, c.NU * CH:] = 0.0
    return m


def make_in_maps(c, n_cores, inputs):
    """Shard full inputs over cores; returns list of per-core input dicts."""
    x_heads = np.asarray(inputs["x_heads"], np.float32)
    seq_repr = np.asarray(inputs["seq_repr"], np.float32)
    seq_mask = np.asarray(inputs["seq_mask"])
    B = x_heads.shape[0]
    assert B % n_cores == 0
    BL = B // n_cores
    assert BL == c.BL

    shared = {
        "wg": _bf16(inputs["sf_gate_w"]),
        "wv": _bf16(inputs["sf_val_w"]),
        "wo": _bf16(inputs["sf_out_w"]),
        "sfgb": _f32(inputs["sf_gate_b"]),
        "sfvb": _f32(inputs["sf_val_b"]),
        "sfob": _f32(inputs["sf_out_b"]),
        "sfnw": _f32(inputs["sf_norm_w"]),
        "kw": _bf16(inputs["k_w"]),
        "vw": _bf16(inputs["v_w"]),
        "kb": _f32(inputs["k_b"]),
        "vb": _f32(inputs["v_b"]),
        "qnin": _f32(inputs["qm_norm_in_w"]),
        "qnh": _f32(inputs["qm_norm_head_w"]),
        "onw": _f32(inputs["of_norm_w"]),
        "um": np.ascontiguousarray(np.tile(_ui_mask(c), (c.BL, 1))),
        "qgw": _bf16(inputs["qm_gate_w"]),
        "qvw": _bf16(inputs["qm_val_w"]),
        "qow": _bf16(inputs["qm_out_w"]),
        "qgbT": _biasT(inputs["qm_gate_b"], c),
        "qvbT": _biasT(inputs["qm_val_b"], c),
        "qob": np.ascontiguousarray(np.tile(_f32(inputs["qm_out_b"]).reshape(c.H, 128), (c.BL, 1))),
        "ogw": _bf16(inputs["of_gate_w"]),
        "ovw": _bf16(inputs["of_val_w"]),
        "oow": _bf16(inputs["of_out_w"]),
        "ogbT": _biasT(inputs["of_gate_b"], c),
        "ovbT": _biasT(inputs["of_val_b"], c),
        "oob": np.ascontiguousarray(np.tile(_f32(inputs["of_out_b"]).reshape(c.H, 128), (c.BL, 1))),
    }

    in_maps = []
    for core in range(n_cores):
        b0 = core * BL
        mask_add = np.where(seq_mask[b0:b0 + BL], 0.0, -1e30).astype(np.float32)  # [BL, T]
        maskT = np.ascontiguousarray(
            mask_add.reshape(c.BL, c.TCB, 128).transpose(2, 1, 0))  # [128, TCB, BL]
        m = dict(shared)
        m["xseq"] = np.ascontiguousarray(seq_repr[b0:b0 + BL].reshape(c.TT, c.D))
        m["xh"] = np.ascontiguousarray(x_heads[b0:b0 + BL].reshape(c.BH, 128))
        m["maskT"] = maskT
        in_maps.append(m)
    return in_maps


_NC_CACHE = {}


def _get_nc(c):
    key = (c.BL, c.T, c.D, c.H, c.HF, c.HQ)
    if key not in _NC_CACHE:
        _NC_CACHE[key] = build_nc(c)
    return _NC_CACHE[key]


def run_cores(c, in_maps, core_ids):
    nc = _get_nc(c)
    res = run_bass_kernel_spmd(nc, in_maps, core_ids=core_ids)
    return res


def kernel(**inputs):
    c = FULL
    in_maps = make_in_maps(c, N_CORES, inputs)
    res = run_cores(c, in_maps, list(range(N_CORES)))
    o = np.concatenate(
        [r["oout"].reshape(c.BL, c.H, 128) for r in res.results], axis=0)
    seqh = np.concatenate(
        [r["seqh"].reshape(c.BL, c.T, c.D) for r in res.results], axis=0)
    return (np.asarray(o, np.float32), np.asarray(seqh, np.float32))


# revision 34
# speedup vs baseline: 1.9793x; 1.9793x over previous
"""Trainium2 Bass kernel for nn_MixFormerBlock (8 NeuronCores, data-parallel over batch).

Strategy (per core, 2 of 16 batches => 2048 tokens):
  - seq FFN (dominant: 3 x [2048x2048]@[2048x8192]-class matmuls) in bf16 on PE:
      rmsnorm -> sn (bf16, spilled to HBM) -> per token-quarter: DMA-transpose to snT,
      G/V matmuls (weights stationary, tokens moving), silu*val -> H^T resident in SBUF,
      out matmuls (H^T stationary, Wo moving) -> +residual -> seq_hidden (fp32 out,
      bf16 copy spilled to HBM for attention).
  - attention: per head, DMA-transpose seq_hidden head-slice, keys^T/vals matmuls,
    scores^T via PE (token-partition layout), exp on ACT, denominators via ones-matmul,
    z^T accumulated into one PSUM tile, PE-transpose back.
  - QueryMixer / OutputFusion small per-head swiglus batched over (batch, head) columns.
No collectives: pure SPMD data parallelism; host gathers per-core outputs.
"""

import os
import sys

for _p in ("/opt/trn_rl_repo", "/root/.axon_site/_ro/trn_rl_repo"):
    if os.path.isdir(_p) and _p not in sys.path:
        sys.path.insert(0, _p)

import math
from contextlib import ExitStack
import numpy as np
import ml_dtypes

import concourse.bass as bass
import concourse.mybir as mybir
import concourse.tile as tile
from concourse import bacc
from concourse.bass_utils import run_bass_kernel_spmd
from concourse.masks import make_identity

F32 = mybir.dt.float32
BF16 = mybir.dt.bfloat16
AX = mybir.AxisListType.X
AF = mybir.ActivationFunctionType
OP = mybir.AluOpType
EPS = 1e-8


class Cfg:
    def __init__(self, BL, T, D, H, HF, HQ, NU):
        self.BL = BL          # local batches per core
        self.T = T            # seq len per batch
        self.D = D            # model dim (= H*128)
        self.H = H            # heads
        self.HF = HF          # seq FFN hidden
        self.HQ = HQ          # per-head FFN hidden
        self.NU = NU          # num user heads (mask only, host side)
        self.HD = 128
        self.TT = BL * T      # local tokens
        self.KC = D // 128    # D chunks
        self.NCF = HF // 128  # HF chunks
        self.KQ = HQ // 128   # HQ chunks
        self.BH = BL * H
        self.TCB = T // 128   # token chunks per batch
        self.GTC = self.TT // 128
        self.TQ = min(512, self.T)    # tokens per FFN chunk ("quarter")
        self.NQ = self.TT // self.TQ
        self.DHW = min(512, D)        # out-matmul rhs width
        self.NDH = D // self.DHW
        assert self.T % self.TQ == 0 and self.TQ % 128 == 0
        assert D == H * 128


FULL = Cfg(BL=2, T=1024, D=2048, H=16, HF=8192, HQ=512, NU=4)
N_CORES = 8
B_FULL, T_FULL, D_FULL = 16, 1024, 2048


def _rms_small(nc, pool, psum, x_sb, w_rep, eps_col, out_dt, name):
    """rmsnorm over free dim (128) of x_sb [P,128] f32; returns [P,128] out_dt."""
    P = x_sb.shape[0]
    sq = pool.tile([P, 128], F32, name=f"{name}_sq")
    nc.vector.tensor_mul(sq, x_sb, x_sb)
    s = pool.tile([P, 1], F32, name=f"{name}_s")
    nc.vector.reduce_sum(s, sq, axis=AX)
    nc.scalar.activation(s, s, AF.Sqrt, bias=eps_col[:P], scale=1.0 / 128)
    nc.vector.reciprocal(s, s)
    tmp = pool.tile([P, 128], F32, name=f"{name}_tmp")
    nc.vector.tensor_scalar_mul(tmp, x_sb, s)
    out = pool.tile([P, 128], out_dt, name=f"{name}_out")
    nc.vector.tensor_tensor(out, tmp, w_rep[:P], OP.mult)
    return out


def _swiglu_T(nc, c, pool, psum, idn_f32, xT_bf, gw_sb, vw_sb, ow_sb, gbT, vbT, name):
    """Per-head swiglu, batched over (b,h) columns.

    xT_bf: [128 d, BH] bf16 (columns ordered b*H+h).
    gw_sb/vw_sb: [128 d, H, HQ] bf16;  ow_sb: [128 k, H, KQ, 128] bf16.
    gbT/vbT: [128 k, KQ, H] f32.
    Returns osw rows [BH, 128] f32 in PSUM (caller consumes immediately).
    """
    KQ, H, BL, BH = c.KQ, c.H, c.BL, c.BH
    gT = pool.tile([128, KQ, BL, H], F32, name=f"{name}_gT")
    vT = pool.tile([128, KQ, BL, H], F32, name=f"{name}_vT")
    xT_by_h = xT_bf.rearrange("p (b h) -> p h b", h=H)
    for h in range(H):
        gw_h = pool.tile([128, c.HQ], BF16, name=f"{name}_gwh", tag=f"{name}_wst")
        nc.sync.dma_start(out=gw_h, in_=gw_sb[:, h, :])
        vw_h = pool.tile([128, c.HQ], BF16, name=f"{name}_vwh", tag=f"{name}_wst")
        nc.sync.dma_start(out=vw_h, in_=vw_sb[:, h, :])
        gv_ps = psum.tile([128, 2, KQ, BL], F32, name=f"{name}_gvps")
        for kc in range(KQ):
            nc.tensor.matmul(gv_ps[:, 0, kc, :], lhsT=gw_h[:, kc * 128:(kc + 1) * 128],
                             rhs=xT_by_h[:, h, :], start=True, stop=True)
            nc.tensor.matmul(gv_ps[:, 1, kc, :], lhsT=vw_h[:, kc * 128:(kc + 1) * 128],
                             rhs=xT_by_h[:, h, :], start=True, stop=True)
        nc.vector.tensor_copy(gT[:, :, :, h], gv_ps[:, 0, :, :])
        nc.vector.tensor_copy(vT[:, :, :, h], gv_ps[:, 1, :, :])
    shp = [128, KQ, BL, H]
    nc.vector.tensor_tensor(gT, gT, gbT[:, :, None, :].to_broadcast(shp), OP.add)
    gs = pool.tile(shp, BF16, name=f"{name}_gs")
    nc.scalar.activation(gs, gT, AF.Silu)
    vs = pool.tile(shp, BF16, name=f"{name}_vs")
    nc.vector.tensor_tensor(vs, vT, vbT[:, :, None, :].to_broadcast(shp), OP.add)
    hT = pool.tile(shp, BF16, name=f"{name}_hT")
    nc.vector.tensor_mul(hT, gs, vs)
    # out matmuls, transposed: oT[e, bh] accumulated per column
    oT_ps = psum.tile([128, BH], F32, name=f"{name}_oTps")
    for h in range(H):
        ow_h = pool.tile([128, KQ, 128], BF16, name=f"{name}_owh", tag=f"{name}_wst")
        nc.sync.dma_start(out=ow_h, in_=ow_sb[:, h, :, :])
        for kc in range(KQ):
            for b in range(BL):
                nc.tensor.matmul(oT_ps[:, b * H + h: b * H + h + 1],
                                 lhsT=ow_h[:, kc, :],
                                 rhs=hT[:, kc, b, h:h + 1],
                                 start=(kc == 0), stop=(kc == KQ - 1),
                                 skip_group_check=True)
    oT_sb = pool.tile([128, BH], F32, name=f"{name}_oTsb")
    nc.vector.tensor_copy(oT_sb, oT_ps)
    o_ps = psum.tile([BH, 128], F32, name=f"{name}_ops")
    nc.tensor.transpose(o_ps, oT_sb, idn_f32)
    return o_ps


def build_nc(c: Cfg):
    nc = bacc.Bacc(None, target_bir_lowering=False)

    # ---- DRAM I/O ----
    xseq = nc.dram_tensor("xseq", [c.TT, c.D], F32, kind="ExternalInput")[:]
    xh = nc.dram_tensor("xh", [c.BH, 128], F32, kind="ExternalInput")[:]
    maskT = nc.dram_tensor("maskT", [128, c.TCB, c.BL], F32, kind="ExternalInput")[:]
    wg = nc.dram_tensor("wg", [c.D, c.HF], BF16, kind="ExternalInput")[:]
    wv = nc.dram_tensor("wv", [c.D, c.HF], BF16, kind="ExternalInput")[:]
    wo = nc.dram_tensor("wo", [c.HF, c.D], BF16, kind="ExternalInput")[:]
    sfgb = nc.dram_tensor("sfgb", [c.HF], F32, kind="ExternalInput")[:]
    sfvb = nc.dram_tensor("sfvb", [c.HF], F32, kind="ExternalInput")[:]
    sfob = nc.dram_tensor("sfob", [c.D], BF16, kind="ExternalInput")[:]
    sfnw = nc.dram_tensor("sfnw", [c.D], BF16, kind="ExternalInput")[:]
    kw = nc.dram_tensor("kw", [c.H, 128, 128], BF16, kind="ExternalInput")[:]
    vw = nc.dram_tensor("vw", [c.H, 128, 128], BF16, kind="ExternalInput")[:]
    kb = nc.dram_tensor("kb", [c.H, 128], F32, kind="ExternalInput")[:]
    vb = nc.dram_tensor("vb", [c.H, 128], BF16, kind="ExternalInput")[:]
    # qm / of small weights
    qnin = nc.dram_tensor("qnin", [128], F32, kind="ExternalInput")[:]
    qnh = nc.dram_tensor("qnh", [128], F32, kind="ExternalInput")[:]
    onw = nc.dram_tensor("onw", [128], F32, kind="ExternalInput")[:]
    um = nc.dram_tensor("um", [c.BH, 128], F32, kind="ExternalInput")[:]
    qgw = nc.dram_tensor("qgw", [c.H, 128, c.HQ], BF16, kind="ExternalInput")[:]
    qvw = nc.dram_tensor("qvw", [c.H, 128, c.HQ], BF16, kind="ExternalInput")[:]
    qow = nc.dram_tensor("qow", [c.H, c.HQ, 128], BF16, kind="ExternalInput")[:]
    qgbT = nc.dram_tensor("qgbT", [128, c.KQ, c.H], F32, kind="ExternalInput")[:]
    qvbT = nc.dram_tensor("qvbT", [128, c.KQ, c.H], F32, kind="ExternalInput")[:]
    qob = nc.dram_tensor("qob", [c.BH, 128], F32, kind="ExternalInput")[:]
    ogw = nc.dram_tensor("ogw", [c.H, 128, c.HQ], BF16, kind="ExternalInput")[:]
    ovw = nc.dram_tensor("ovw", [c.H, 128, c.HQ], BF16, kind="ExternalInput")[:]
    oow = nc.dram_tensor("oow", [c.H, c.HQ, 128], BF16, kind="ExternalInput")[:]
    ogbT = nc.dram_tensor("ogbT", [128, c.KQ, c.H], F32, kind="ExternalInput")[:]
    ovbT = nc.dram_tensor("ovbT", [128, c.KQ, c.H], F32, kind="ExternalInput")[:]
    oob = nc.dram_tensor("oob", [c.BH, 128], F32, kind="ExternalInput")[:]

    seqh = nc.dram_tensor("seqh", [c.TT, c.D], F32, kind="ExternalOutput")[:]
    oout = nc.dram_tensor("oout", [c.BH, 128], F32, kind="ExternalOutput")[:]

    with tile.TileContext(nc) as tc:
        with tc.tile_pool(name="dram", bufs=1, space="DRAM") as dram, \
             tc.tile_pool(name="singles", bufs=1) as sg:

            sn_dram = dram.tile([c.TT, c.D], BF16)
            shb_dram = dram.tile([c.TT, c.D], BF16)
            qm_dram = dram.tile([c.BH, 128], F32)
            rinv_dram = dram.tile([c.BH], F32)

            # ---- persistent small tiles ----
            idn_bf = sg.tile([128, 128], BF16)
            make_identity(nc, idn_bf)
            idn_f32 = sg.tile([128, 128], F32)
            make_identity(nc, idn_f32)
            eps_col = sg.tile([128, 1], F32)
            nc.vector.memset(eps_col, EPS)
            ones_bf = sg.tile([128, 1], BF16)
            nc.vector.memset(ones_bf, 1.0)

            sfnw_sb = sg.tile([128, c.D], BF16)
            nc.sync.dma_start(out=sfnw_sb, in_=sfnw[None, :].to_broadcast([128, c.D]))
            sfob_sb = sg.tile([128, c.D], BF16)
            nc.sync.dma_start(out=sfob_sb, in_=sfob[None, :].to_broadcast([128, c.D]))
            gb_sb = sg.tile([128, c.NCF], F32)
            nc.sync.dma_start(out=gb_sb, in_=sfgb.rearrange("(n p) -> p n", p=128))
            vbf_sb = sg.tile([128, c.NCF], F32)
            nc.sync.dma_start(out=vbf_sb, in_=sfvb.rearrange("(n p) -> p n", p=128))

            kw_sb = sg.tile([128, c.H, 128], BF16)
            nc.sync.dma_start(out=kw_sb, in_=kw.rearrange("h d e -> d h e"))
            vw_sb = sg.tile([128, c.H, 128], BF16)
            nc.sync.dma_start(out=vw_sb, in_=vw.rearrange("h d e -> d h e"))
            kb_sb = sg.tile([128, c.H], F32)
            nc.sync.dma_start(out=kb_sb, in_=kb.rearrange("h e -> e h"))
            vbh_sb = sg.tile([128, c.H, 128], BF16)
            nc.sync.dma_start(out=vbh_sb, in_=vb[None, :, :].to_broadcast([128, c.H, 128]))
            maskT_sb = sg.tile([128, c.TCB, c.BL], F32)
            nc.sync.dma_start(out=maskT_sb, in_=maskT)

            qnin_sb = sg.tile([128, 128], F32)
            nc.sync.dma_start(out=qnin_sb, in_=qnin[None, :].to_broadcast([128, 128]))
            qnh_sb = sg.tile([128, 128], F32)
            nc.sync.dma_start(out=qnh_sb, in_=qnh[None, :].to_broadcast([128, 128]))
            onw_sb = sg.tile([128, 128], F32)
            nc.sync.dma_start(out=onw_sb, in_=onw[None, :].to_broadcast([128, 128]))
            um_sb = sg.tile([c.BH, 128], F32)
            nc.sync.dma_start(out=um_sb, in_=um)
            qob_sb = sg.tile([c.BH, 128], F32)
            nc.sync.dma_start(out=qob_sb, in_=qob)
            oob_sb = sg.tile([c.BH, 128], F32)
            nc.sync.dma_start(out=oob_sb, in_=oob)

            # persistent cross-phase results
            q_all = sg.tile([c.BH, 128], F32)       # q (unscaled)
            qTs = sg.tile([128, c.BH], BF16)        # q^T * 1/sqrt(128)
            z_all = sg.tile([c.BH, 128], F32)       # z = attn@vals + q
            expT = sg.tile([128, c.BL, c.TCB, c.H], BF16)
            rinv = sg.tile([1, c.BL, c.H], F32)
            zT_acc = sg.tile([128, c.BH], F32)      # flash-attn z accumulator
            nc.vector.memset(zT_acc, 0.0)
            den_acc = sg.tile([1, c.BL, c.H, c.TCB], F32)
            nc.vector.memset(den_acc, 0.0)

            # early FFN pools (addresses below the QM pools, so FFN start
            # never waits on QM's SBUF space)
            es = ExitStack()
            ffh = es.enter_context(tc.tile_pool(name="ffh", bufs=1))
            ffw = es.enter_context(tc.tile_pool(name="ffw", bufs=2))
            ffs = es.enter_context(tc.tile_pool(name="ffs", bufs=2))
            ffr = es.enter_context(tc.tile_pool(name="ffr", bufs=2))
            ffgv = es.enter_context(tc.tile_pool(name="ffgv", bufs=2))

            # ================= QueryMixer =================
            with tc.tile_pool(name="qmw", bufs=1) as qmw, \
                 tc.tile_pool(name="qmp", bufs=2) as qmp, \
                 tc.tile_pool(name="qmps", bufs=1, space="PSUM") as qmps:
                qgw_sb = qgw.rearrange("h d k -> d h k")
                qvw_sb = qvw.rearrange("h d k -> d h k")
                qow_sb = qow.rearrange("h (kc p) e -> p h kc e", p=128)
                qgbT_sb = qmw.tile([128, c.KQ, c.H], F32)
                nc.sync.dma_start(out=qgbT_sb, in_=qgbT)
                qvbT_sb = qmw.tile([128, c.KQ, c.H], F32)
                nc.sync.dma_start(out=qvbT_sb, in_=qvbT)

                xh_sb = qmp.tile([c.BH, 128], F32, name="xh_sb")
                nc.sync.dma_start(out=xh_sb, in_=xh)
                nin = _rms_small(nc, qmp, qmps, xh_sb, qnin_sb, eps_col, F32, "qmn")
                nc.sync.dma_start(out=qm_dram[:], in_=nin)
                mixed = qmp.tile([c.BH, 128], F32, name="mixed")
                for b in range(c.BL):
                    nc.sync.dma_start(
                        out=mixed[b * c.H:(b + 1) * c.H].rearrange("p (j ch) -> p j ch", j=c.H),
                        in_=qm_dram[b * c.H:(b + 1) * c.H].rearrange("j (i ch) -> i j ch", i=c.H),
                    )
                p_sb = qmp.tile([c.BH, 128], F32, name="p_sb")
                nc.vector.tensor_mul(p_sb, mixed, um_sb)
                nc.vector.tensor_add(p_sb, p_sb, xh_sb)
                pn = _rms_small(nc, qmp, qmps, p_sb, qnh_sb, eps_col, BF16, "qmh")
                pnT_ps = qmps.tile([128, c.BH], BF16, name="pnT_ps")
                nc.tensor.transpose(pnT_ps, pn, idn_bf[:c.BH, :c.BH])
                pnT = qmp.tile([128, c.BH], BF16, name="pnT")
                nc.vector.tensor_copy(pnT, pnT_ps)
                osw_ps = _swiglu_T(nc, c, qmp, qmps, idn_f32, pnT,
                                   qgw_sb, qvw_sb, qow_sb, qgbT_sb, qvbT_sb, "qm")
                nc.vector.tensor_add(q_all, osw_ps, p_sb)
                nc.vector.tensor_add(q_all, q_all, qob_sb)
                qs_bf = qmp.tile([c.BH, 128], BF16, name="qs_bf")
                nc.vector.tensor_scalar_mul(qs_bf, q_all, 1.0 / math.sqrt(128.0))
                qTs_ps = qmps.tile([128, c.BH], BF16, name="qTs_ps")
                nc.tensor.transpose(qTs_ps, qs_bf, idn_bf[:c.BH, :c.BH])
                nc.vector.tensor_copy(qTs, qTs_ps)

            # ====== seq FFN (quarters) with attention interleaved ======
            with tc.tile_pool(name="ffwo", bufs=6) as ffwo, \
                 tc.tile_pool(name="ffe", bufs=2) as ffe, \
                 tc.tile_pool(name="atp", bufs=2) as atp:

                Hq = ffh.tile([128, c.NCF, c.TQ], BF16)
                wg_r = wg.rearrange("(kc p) f -> p kc f", p=128)
                wv_r = wv.rearrange("(kc p) f -> p kc f", p=128)
                wo_r = wo.rearrange("(kc p) d -> p kc d", p=128)
                NTC = c.TQ // 128

                # ---- attention for one quarter's tokens (keys/vals/scores/z) ----
                def emit_att(q, atps):
                    t0 = q * c.TQ
                    g0 = q * NTC          # first global 128-chunk of this quarter
                    bq = (q * c.TQ) // c.T
                    tcb0 = g0 % c.TCB
                    exp_dst = lambda h: expT[:, bq, tcb0:tcb0 + NTC, h]
                    mask_sl = maskT_sb[:, tcb0:tcb0 + NTC, bq]
                    for h in range(c.H):
                        shT_qh = atp.tile([128, c.TQ], BF16, name="shT_qh")
                        nc.sync.dma_start_transpose(
                            shT_qh, shb_dram[t0:t0 + c.TQ, h * 128:(h + 1) * 128])
                        kt_ps = atps.tile([128, c.TQ], F32, name="kt_ps", tag="atr")
                        nc.tensor.matmul(kt_ps, lhsT=kw_sb[:, h, :], rhs=shT_qh,
                                         start=True, stop=True)
                        kt_sb = atp.tile([128, c.TQ], BF16, name="kt_sb")
                        nc.vector.tensor_scalar(kt_sb, kt_ps, scalar1=kb_sb[:, h:h + 1],
                                                scalar2=None, op0=OP.add)
                        va_ps = atps.tile([128, NTC, 128], F32, name="va_ps", tag="atr")
                        for j in range(NTC):
                            nc.tensor.matmul(va_ps[:, j, :],
                                             lhsT=shT_qh[:, j * 128:(j + 1) * 128],
                                             rhs=vw_sb[:, h, :], start=True, stop=True)
                        va_sb = atp.tile([128, NTC, 128], BF16, name="va_sb")
                        nc.vector.tensor_tensor(
                            va_sb, va_ps,
                            vbh_sb[:, h, :][:, None, :].to_broadcast([128, NTC, 128]), OP.add)
                        st_ps = atps.tile([128, NTC], F32, name="st_ps", tag="atr")
                        for j in range(NTC):
                            nc.tensor.matmul(st_ps[:, j:j + 1],
                                             lhsT=kt_sb[:, j * 128:(j + 1) * 128],
                                             rhs=qTs[:, bq * c.H + h:bq * c.H + h + 1],
                                             start=True, stop=True)
                        st_sb = atp.tile([128, NTC], F32, name="st_sb")
                        nc.vector.tensor_tensor(st_sb, st_ps, mask_sl, OP.add)
                        nc.scalar.activation(exp_dst(h), st_sb, AF.Exp)
                        # denominators: column sums via ones-matmul -> SBUF slice
                        dn_ps = atps.tile([1, NTC], F32, name="dn_ps", tag="atr")
                        nc.tensor.matmul(dn_ps, lhsT=ones_bf, rhs=exp_dst(h),
                                         start=True, stop=True)
                        nc.vector.tensor_copy(den_acc[:, bq, h, tcb0:tcb0 + NTC], dn_ps)
                        # z partial for this quarter -> accumulate into zT_acc column
                        zq_ps = atps.tile([128, 1], F32, name="zq_ps", tag="atr")
                        for j in range(NTC):
                            nc.tensor.matmul(zq_ps,
                                             lhsT=va_sb[:, j, :],
                                             rhs=expT[:, bq, tcb0 + j, h:h + 1],
                                             start=(j == 0), stop=(j == NTC - 1))
                        col = slice(bq * c.H + h, bq * c.H + h + 1)
                        nc.vector.tensor_add(zT_acc[:, col], zT_acc[:, col], zq_ps)



                for q in range(c.NQ):
                    t0 = q * c.TQ
                    # rmsnorm for this quarter's tokens -> sn_dram
                    for tcq in range(NTC):
                        tt = q * NTC + tcq
                        xt = ffr.tile([128, c.D], F32, name="r_xt")
                        nc.sync.dma_start(out=xt, in_=xseq[tt * 128:(tt + 1) * 128, :])
                        snt = ffr.tile([128, c.D], BF16, name="r_snt", bufs=2)
                        sq = ffr.tile([128, c.D], F32, name="r_sq", bufs=1)
                        s = ffr.tile([128, 1], F32, name="r_s")
                        nc.vector.tensor_mul(sq, xt, xt)
                        nc.vector.reduce_sum(s, sq, axis=AX)
                        nc.scalar.activation(s, s, AF.Sqrt, bias=eps_col, scale=1.0 / c.D)
                        nc.vector.reciprocal(s, s)
                        nc.vector.tensor_scalar_mul(sq, xt, s)
                        nc.vector.tensor_tensor(snt, sq, sfnw_sb, OP.mult)
                        nc.sync.dma_start(out=sn_dram[tt * 128:(tt + 1) * 128, :], in_=snt)
                    # transpose sn chunk -> snT_q
                    snT_q = ffs.tile([128, c.KC, c.TQ], BF16, name="snT_q")
                    for dc in range(c.KC):
                        nc.sync.dma_start_transpose(
                            snT_q[:, dc, :],
                            sn_dram[t0:t0 + c.TQ, dc * 128:(dc + 1) * 128])
                    # G/V + H  (PSUM pool scoped to this phase)
                    with tc.tile_pool(name="gvps", bufs=2, space="PSUM") as gvps, \
                         tc.tile_pool(name="atps", bufs=2, space="PSUM") as atps_q:
                        for ncf in range(c.NCF):
                            wg_sl = ffw.tile([128, c.KC, 128], BF16, name="wg_sl")
                            nc.sync.dma_start(out=wg_sl, in_=wg_r[:, :, ncf * 128:(ncf + 1) * 128])
                            wv_sl = ffw.tile([128, c.KC, 128], BF16, name="wv_sl")
                            nc.sync.dma_start(out=wv_sl, in_=wv_r[:, :, ncf * 128:(ncf + 1) * 128])
                            g_ps = gvps.tile([128, c.TQ], F32, name="g_ps")
                            for kc in range(c.KC):
                                nc.tensor.matmul(g_ps, lhsT=wg_sl[:, kc, :], rhs=snT_q[:, kc, :],
                                                 start=(kc == 0), stop=(kc == c.KC - 1))
                            gs = ffgv.tile([128, c.TQ], BF16, name="gs")
                            nc.scalar.activation(gs, g_ps, AF.Silu,
                                                 bias=gb_sb[:, ncf:ncf + 1], scale=1.0)
                            v_ps = gvps.tile([128, c.TQ], F32, name="v_ps")
                            for kc in range(c.KC):
                                nc.tensor.matmul(v_ps, lhsT=wv_sl[:, kc, :], rhs=snT_q[:, kc, :],
                                                 start=(kc == 0), stop=(kc == c.KC - 1))
                            vs = ffgv.tile([128, c.TQ], BF16, name="vs")
                            nc.vector.tensor_scalar(vs, v_ps, scalar1=vbf_sb[:, ncf:ncf + 1],
                                                    scalar2=None, op0=OP.add)
                            nc.vector.tensor_mul(Hq[:, ncf, :], gs, vs)
                            if q > 0 and ncf == c.NCF // 2:
                                emit_att(q - 1, atps_q)
                    # out matmuls: [128, DW2] psum per token-tile (2 banks), 2 MMs/LDW
                    DW2 = c.DHW  # bisect: single-slice groups
                    with tc.tile_pool(name="outps", bufs=1, space="PSUM") as outps:
                        for dh in range(c.D // DW2):
                            d0 = dh * DW2
                            out_tiles = []
                            for tcq in range(NTC):
                                out_tiles.append(outps.tile([128, DW2], F32, name=f"out_ps{tcq}"))
                            for kc in range(c.NCF):
                                wo_sl = ffwo.tile([128, DW2], BF16, name="wo_sl")
                                nc.sync.dma_start(out=wo_sl, in_=wo_r[:, kc, d0:d0 + DW2])
                                for tcq in range(NTC):
                                    for w2 in range(DW2 // c.DHW):
                                        nc.tensor.matmul(
                                            out_tiles[tcq][:, w2 * c.DHW:(w2 + 1) * c.DHW],
                                            lhsT=Hq[:, kc, tcq * 128:(tcq + 1) * 128],
                                            rhs=wo_sl[:, w2 * c.DHW:(w2 + 1) * c.DHW],
                                            start=(kc == 0), stop=(kc == c.NCF - 1),
                                            skip_group_check=True)
                            for tcq in range(NTC):
                                rows = slice(t0 + tcq * 128, t0 + (tcq + 1) * 128)
                                xre = ffe.tile([128, DW2], F32, name="xre")
                                nc.sync.dma_start(out=xre, in_=xseq[rows, d0:d0 + DW2])
                                t1 = ffe.tile([128, DW2], F32, name="t1")
                                nc.vector.tensor_add(t1, out_tiles[tcq], xre)
                                nc.vector.tensor_tensor(
                                    t1, t1, sfob_sb[:, d0:d0 + DW2], OP.add)
                                nc.sync.dma_start(out=seqh[rows, d0:d0 + DW2], in_=t1)
                                shb = ffe.tile([128, DW2], BF16, name="shb")
                                nc.scalar.activation(shb, t1, AF.Copy)
                                nc.sync.dma_start(out=shb_dram[rows, d0:d0 + DW2], in_=shb)

                # ---- attention tail: last quarter + denominators -> rinv, z ----
                with tc.tile_pool(name="tailps", bufs=2, space="PSUM") as tailps:
                    emit_att(c.NQ - 1, tailps)
                    rsum = atp.tile([1, c.BL, c.H], F32, name="rsum")
                    nc.vector.reduce_sum(rsum, den_acc, axis=AX)
                    nc.vector.reciprocal(rinv, rsum)
                    nc.sync.dma_start(out=rinv_dram[:], in_=rinv.rearrange("p b h -> p (b h)"))
                    rinv_col = atp.tile([c.BH, 1], F32, name="rinv_col")
                    nc.sync.dma_start(out=rinv_col, in_=rinv_dram[:, None])
                    z_ps = tailps.tile([c.BH, 128], F32, name="z_ps", tag="atr")
                    nc.tensor.transpose(z_ps, zT_acc, idn_f32)
                    nc.vector.tensor_scalar(z_all, z_ps, scalar1=rinv_col,
                                            scalar2=None, op0=OP.mult)
                    nc.vector.tensor_add(z_all, z_all, q_all)

            es.close()

            # ================= OutputFusion =================
            with tc.tile_pool(name="ofw", bufs=1) as ofw, \
                 tc.tile_pool(name="ofp", bufs=2) as ofp, \
                 tc.tile_pool(name="ofps", bufs=1, space="PSUM") as ofps:
                ogw_sb = ogw.rearrange("h d k -> d h k")
                ovw_sb = ovw.rearrange("h d k -> d h k")
                oow_sb = oow.rearrange("h (kc p) e -> p h kc e", p=128)
                ogbT_sb = ofw.tile([128, c.KQ, c.H], F32)
                nc.sync.dma_start(out=ogbT_sb, in_=ogbT)
                ovbT_sb = ofw.tile([128, c.KQ, c.H], F32)
                nc.sync.dma_start(out=ovbT_sb, in_=ovbT)

                zn = _rms_small(nc, ofp, ofps, z_all, onw_sb, eps_col, BF16, "ofn")
                znT_ps = ofps.tile([128, c.BH], BF16, name="znT_ps")
                nc.tensor.transpose(znT_ps, zn, idn_bf[:c.BH, :c.BH])
                znT = ofp.tile([128, c.BH], BF16, name="znT")
                nc.vector.tensor_copy(znT, znT_ps)
                osw_ps = _swiglu_T(nc, c, ofp, ofps, idn_f32, znT,
                                   ogw_sb, ovw_sb, oow_sb, ogbT_sb, ovbT_sb, "of")
                o_sb = ofp.tile([c.BH, 128], F32, name="o_sb")
                nc.vector.tensor_add(o_sb, osw_ps, z_all)
                nc.vector.tensor_add(o_sb, o_sb, oob_sb)
                nc.sync.dma_start(out=oout, in_=o_sb)

    nc.finalize()
    return nc


# ---------------- host side ----------------

def _bf16(a):
    return np.asarray(a, dtype=np.float32).astype(ml_dtypes.bfloat16)


def _f32(a):
    return np.ascontiguousarray(np.asarray(a, dtype=np.float32))


def _biasT(b, c):
    # [H, HQ] -> [128, KQ, H]
    b = np.asarray(b, np.float32)
    return np.ascontiguousarray(b.T.reshape(c.KQ, 128, c.H).transpose(1, 0, 2))


def _ui_mask(c):
    CH = 128 // c.H
    m = np.ones((c.H, 128), dtype=np.float32)
    m[:c.NU, c.NU * CH:] = 0.0
    return m


def make_in_maps(c, n_cores, inputs):
    """Shard full inputs over cores; returns list of per-core input dicts."""
    x_heads = np.asarray(inputs["x_heads"], np.float32)
    seq_repr = np.asarray(inputs["seq_repr"], np.float32)
    seq_mask = np.asarray(inputs["seq_mask"])
    B = x_heads.shape[0]
    assert B % n_cores == 0
    BL = B // n_cores
    assert BL == c.BL

    shared = {
        "wg": _bf16(inputs["sf_gate_w"]),
        "wv": _bf16(inputs["sf_val_w"]),
        "wo": _bf16(inputs["sf_out_w"]),
        "sfgb": _f32(inputs["sf_gate_b"]),
        "sfvb": _f32(inputs["sf_val_b"]),
        "sfob": _bf16(inputs["sf_out_b"]),
        "sfnw": _bf16(inputs["sf_norm_w"]),
        "kw": _bf16(inputs["k_w"]),
        "vw": _bf16(inputs["v_w"]),
        "kb": _f32(inputs["k_b"]),
        "vb": _bf16(inputs["v_b"]),
        "qnin": _f32(inputs["qm_norm_in_w"]),
        "qnh": _f32(inputs["qm_norm_head_w"]),
        "onw": _f32(inputs["of_norm_w"]),
        "um": np.ascontiguousarray(np.tile(_ui_mask(c), (c.BL, 1))),
        "qgw": _bf16(inputs["qm_gate_w"]),
        "qvw": _bf16(inputs["qm_val_w"]),
        "qow": _bf16(inputs["qm_out_w"]),
        "qgbT": _biasT(inputs["qm_gate_b"], c),
        "qvbT": _biasT(inputs["qm_val_b"], c),
        "qob": np.ascontiguousarray(np.tile(_f32(inputs["qm_out_b"]).reshape(c.H, 128), (c.BL, 1))),
        "ogw": _bf16(inputs["of_gate_w"]),
        "ovw": _bf16(inputs["of_val_w"]),
        "oow": _bf16(inputs["of_out_w"]),
        "ogbT": _biasT(inputs["of_gate_b"], c),
        "ovbT": _biasT(inputs["of_val_b"], c),
        "oob": np.ascontiguousarray(np.tile(_f32(inputs["of_out_b"]).reshape(c.H, 128), (c.BL, 1))),
    }

    in_maps = []
    for core in range(n_cores):
        b0 = core * BL
        mask_add = np.where(seq_mask[b0:b0 + BL], 0.0, -1e30).astype(np.float32)  # [BL, T]
        maskT = np.ascontiguousarray(
            mask_add.reshape(c.BL, c.TCB, 128).transpose(2, 1, 0))  # [128, TCB, BL]
        m = dict(shared)
        m["xseq"] = np.ascontiguousarray(seq_repr[b0:b0 + BL].reshape(c.TT, c.D))
        m["xh"] = np.ascontiguousarray(x_heads[b0:b0 + BL].reshape(c.BH, 128))
        m["maskT"] = maskT
        in_maps.append(m)
    return in_maps


_NC_CACHE = {}


def _get_nc(c):
    key = (c.BL, c.T, c.D, c.H, c.HF, c.HQ)
    if key not in _NC_CACHE:
        _NC_CACHE[key] = build_nc(c)
    return _NC_CACHE[key]


def run_cores(c, in_maps, core_ids):
    nc = _get_nc(c)
    res = run_bass_kernel_spmd(nc, in_maps, core_ids=core_ids)
    return res


def kernel(**inputs):
    c = FULL
    in_maps = make_in_maps(c, N_CORES, inputs)
    res = run_cores(c, in_maps, list(range(N_CORES)))
    o = np.concatenate(
        [r["oout"].reshape(c.BL, c.H, 128) for r in res.results], axis=0)
    seqh = np.concatenate(
        [r["seqh"].reshape(c.BL, c.T, c.D) for r in res.results], axis=0)
    return (np.asarray(o, np.float32), np.asarray(seqh, np.float32))
